# revision 13
# baseline (speedup 1.0000x reference)
"""Multi-head causal attention (B=2, S=2048, D=1024, H=16, dh=64) on 8 TRN2 cores.

Strategy
--------
- Shard the 32 (batch, head) pairs across 8 cores, 4 pairs each; pure data
  parallel, no collectives. Per core: 2 groups of 2 heads packed into the 128
  SBUF partitions (64 rows each).
- All matmuls in bf16 (1 PE col/cycle at any free size). Host pre-scales Q by
  A/8 with A = 128*log2(e), so the PE writes y = A*score into PSUM -- the
  exact unit both exp paths want.
- S^T = K @ Q^T per head via two 64-contraction quadrant matmuls per
  [128k x W<=512q] block (head h uses PE rows 64h..64h+64, its own PSUM bank).
- exp is the bottleneck, so it is split across two engines:
  * ACT: exact exp (scale=1/A) writing bf16.
  * DVE: Schraudolph fast-exp -- int16(y + B) bit-cast as bf16 is
    exp(score)*(1 +- ~3%). One tensor_scalar per clean block; for diagonal
    blocks one scalar_tensor_tensor fuses the +B with a 0/1 causal-mask
    multiply (masked lanes -> int16 0 -> bf16 +0.0), so masking is free.
    The 0/1 tile is one persistent [128, 516] "triangle composite" whose
    column offset serves every diagonal block. Host-side greedy balances
    the two engines per chunk; a ~62%% exact / 38%% fast mix measures
    ~6e-3 max rel err vs the 2e-2 gate.
- P@V is flipped: out[q=128, d=65] = P^T(lhsT) @ [V | 1](rhs), 65 columns per
  (q-block, k-block) pair instead of 128 -- halves P@V PE time. PSUM
  accumulates over k-blocks; the ones-column yields the softmax denominator.
  Output rows are already [q, d], so the host only divides by column 64.
- PSUM: 2 x s_ps [128,1024] (2 banks each) + 2 x o_ps [128,2,512] (2 banks
  each) = exactly 8 banks; every bank has a single PE writer.
- PE emission interleaves each chunk's S^T blocks with slices of the previous
  chunk's P@V so the PE stays busy while ACT/DVE chew on exp.
"""

import os
import sys
from contextlib import ExitStack

import numpy as np

for _p in ("/opt/trn_rl_repo", "/root/.axon_site/_ro/trn_rl_repo"):
    if os.path.isdir(_p) and _p not in sys.path:
        sys.path.insert(0, _p)
        break

import ml_dtypes  # noqa: E402
import concourse.bacc as bacc  # noqa: E402
import concourse.mybir as mybir  # noqa: E402
import concourse.tile as tile  # noqa: E402
from concourse.bass_utils import run_bass_kernel_spmd  # noqa: E402

F32 = mybir.dt.float32
BF16 = mybir.dt.bfloat16
I16 = mybir.dt.int16
EXP = mybir.ActivationFunctionType.Exp
ALU = mybir.AluOpType
BF = ml_dtypes.bfloat16

N_CORES = 8
H = 16
DH = 64
QBLK = 512
KBLK = 128
VW = DH + 1

A_SCALE = 128.0 * np.log2(np.e)  # exp(s) == 2^(s*log2 e); bf16 bits step 1/128
B_MAGIC = 16256.0 - 5.9          # 127<<7 minus Schraudolph bias correction

# per-instruction cost estimates (ns) for the host-side engine balancer
_ACT_NS = lambda w2: w2 * 0.833 + 190.0
_DVE_STT_NS = lambda w2: w2 * 1.042 + 170.0
_DVE_MUL_NS = lambda w2: w2 * 0.26 + 170.0  # bf16 sbuf 4x mode
_COPY_NS = 305.0
PAV_MIN_SLICE = 8  # P@V matmuls interleaved after each S^T block

LAST_RESULTS = None
LAST_NC = None
LAST_INMAPS = None


def _plan(maskb):
    """Per q-chunk block list [(kk, c0, W, dirty)], causal-verified.

    dirty blocks multiply by the triangle composite comp[:, 4:4+W] where
    comp[row, c] = 1 iff c >= row + 4.
    """
    B, S, _ = maskb.shape
    NQ, NK = S // QBLK, S // KBLK
    assert np.array_equal(maskb[0], np.tril(np.ones((S, S), bool))), \
        "kernel specialised to causal mask"
    for b in range(1, B):
        assert np.array_equal(maskb[b], maskb[0])
    plans = []
    for qc in range(NQ):
        blocks = []
        for kk in range(NK):
            c0 = kk * KBLK - qc * QBLK
            if c0 >= QBLK:
                continue  # fully masked
            c0 = max(0, c0)
            W = QBLK - c0
            dirty = kk >= 4 * qc  # leading 128 cols hold the triangle
            blocks.append((kk, c0, W, dirty))
        plans.append(blocks)
    return plans


def _assign(plans):
    """Greedy per-chunk engine assignment. Returns {(qc, kk): eng} with eng in
    {'act', 'dve', 'act+mul'}. Balances estimated ACT vs DVE ns per chunk,
    charging DVE for the previous chunk's P@V copies."""
    assign = {}
    for qc, blocks in enumerate(plans):
        act_t = 0.0
        dve_t = 4 * _COPY_NS  # prev chunk's o_ps copies land here
        # big blocks first so the tail stays balanced
        for kk, c0, W, dirty in sorted(blocks, key=lambda b: -b[2]):
            if dirty:
                cost_d = _DVE_STT_NS(2 * W)
                cost_a = _ACT_NS(2 * W)
                cost_a_mul = _DVE_MUL_NS(2 * min(W, 132))
                if dve_t + cost_d <= max(act_t + cost_a, dve_t + cost_a_mul):
                    assign[(qc, kk)] = 'dve'
                    dve_t += cost_d
                else:
                    assign[(qc, kk)] = 'act+mul'
                    act_t += cost_a
                    dve_t += cost_a_mul
            else:
                if act_t + _ACT_NS(2 * W) <= dve_t + _DVE_STT_NS(2 * W):
                    assign[(qc, kk)] = 'act'
                    act_t += _ACT_NS(2 * W)
                else:
                    assign[(qc, kk)] = 'dve'
                    dve_t += _DVE_STT_NS(2 * W)
    return assign


def _emit_order(blocks, assign, qc):
    """Interleave ACT- and DVE-assigned blocks in PAIRS. With 2 s_ps bufs a
    same-engine run pipelines back-to-back (the gating exp is 2 blocks old),
    but a singly-inserted foreign block makes an engine gate on its own
    previous exp (+~630ns); pairs keep the 2-back slot on the fast path."""
    act = [b for b in blocks if assign[(qc, b[0])] != 'dve']
    dve = [b for b in blocks if assign[(qc, b[0])] == 'dve']
    out = []
    na, nd = len(act), len(dve)
    pa, pd = -(-na // 2), -(-nd // 2)  # pair counts
    ia = id_ = 0
    while ia < na or id_ < nd:
        # proportional merge at pair granularity
        if id_ >= nd or (ia < na and
                         (ia // 2) * max(pd, 1) <= (id_ // 2) * max(pa, 1)):
            out.extend(act[ia:ia + 2]); ia += 2
        else:
            out.extend(dve[id_:id_ + 2]); id_ += 2
    return out


def _build(S, n_groups, plans, assign):
    NQ, NK = S // QBLK, S // KBLK
    nc = bacc.Bacc("TRN2", target_bir_lowering=False, debug=False)
    qt = nc.declare_dram_parameter("qt", [n_groups, 128, S], BF16, isOutput=False)
    kt = nc.declare_dram_parameter("kt", [n_groups, 128, S], BF16, isOutput=False)
    vv = nc.declare_dram_parameter("vv", [n_groups, 128, 2, NK * VW], BF16,
                                   isOutput=False)
    cm = nc.declare_dram_parameter("cm", [128, 516], BF16, isOutput=False)
    ot = nc.declare_dram_parameter("ot", [n_groups, S, 2 * VW], F32, isOutput=True)

    with tile.TileContext(nc) as tc, ExitStack() as ctx:
        qpool = ctx.enter_context(tc.tile_pool(name="qpool", bufs=2))
        kpool = ctx.enter_context(tc.tile_pool(name="kpool", bufs=2))
        vpool = ctx.enter_context(tc.tile_pool(name="vpool", bufs=4))
        cpool = ctx.enter_context(tc.tile_pool(name="cpool", bufs=1))
        ppool = ctx.enter_context(tc.tile_pool(name="ppool", bufs=28))
        obuf = ctx.enter_context(tc.tile_pool(name="obuf", bufs=2))
        spool = ctx.enter_context(tc.tile_pool(name="spool", bufs=2, space="PSUM"))
        opool = ctx.enter_context(tc.tile_pool(name="opool", bufs=2, space="PSUM"))

        # exp-table warm-up at t=0 (~2.7us) overlaps the first input DMAs
        warm = cpool.tile([128, 8], F32)
        nc.vector.memset(warm[:], 0.0)
        nc.scalar.activation(warm[:], warm[:], EXP)

        comp = cpool.tile([128, 516], BF16)

        # pending P@V work: FIFO of per-q-block items, staged as soon as a
        # q-block's last k-block is exp'd, drained in slices between S^T
        # blocks so the PE fills its s_ps-wait gaps.
        pending = {"items": [], "toggle": [0]}

        def emit_pav_slice(pend, budget):
            """Emit up to `budget` P@V matmuls from pending state."""
            done = 0
            while pend["items"] and done < budget:
                qb_j, mms, o_ps, osb, gg, qb_abs, fin = pend["items"][0]
                while mms and done < budget:
                    fn = mms.pop(0)
                    fn()
                    done += 1
                if not mms:
                    # qb complete: copy into the chunk staging tile
                    dst3 = osb[:, qb_j, :].rearrange("p (h w) -> p h w", h=2)
                    nc.vector.tensor_copy(dst3, o_ps[:, :, 0:VW])
                    if fin:
                        # chunk complete: one DMA for all 4 q-blocks
                        qc = qb_abs // 4
                        dst = ot[gg, qc * QBLK:(qc + 1) * QBLK, :].rearrange(
                            "(t p) w -> p t w", t=4)
                        if pend["toggle"][0] & 1:
                            nc.sync.dma_start(dst, osb[:])
                        else:
                            nc.gpsimd.dma_start(dst, osb[:])
                        pend["toggle"][0] += 1
                    pend["items"].pop(0)
            return done

        def stage_qb(p_tiles, qc, j, g, vtile, osb):
            qb_abs = qc * 4 + j
            o_ps = opool.tile([128, 2, QBLK], F32, tag="o", name="o_ps")
            mms = []
            kks = sorted(kk for kk in p_tiles if kk <= qb_abs)
            for ki, kk in enumerate(kks):
                p_t, c0, W = p_tiles[kk]
                for h in range(2):
                    def mk(kk=kk, h=h, j=j, p_t=p_t, c0=c0,
                           o_ps=o_ps, first=(ki == 0),
                           last=(ki == len(kks) - 1), vtile=vtile):
                        nc.tensor.matmul(
                            o_ps[:, h, 0:VW],
                            lhsT=p_t[:, h * QBLK + j * KBLK - c0:
                                     h * QBLK + j * KBLK - c0 + KBLK],
                            rhs=vtile[:, h, kk * VW:kk * VW + VW],
                            start=first, stop=last)
                    mms.append(mk)
            pending["items"].append((j, mms, o_ps, osb, g, qb_abs, j == 3))

        for g in range(n_groups):
            ktile = kpool.tile([128, S], BF16, tag="kt")
            qtile = qpool.tile([128, S], BF16, tag="qt")
            vtile = vpool.tile([128, 2, NK * VW], BF16, tag="vt", name="vtile")
            # flat-cost DMAs: one per tensor per group, split across queues
            nc.sync.dma_start(ktile[:], kt[g])
            nc.gpsimd.dma_start(qtile[:], qt[g])
            if g == 0:
                nc.gpsimd.dma_start(comp[:], cm[:, :])
            nc.sync.dma_start(vtile[:], vv[g])

            for qc in range(NQ):
                blocks = _emit_order(plans[qc], assign, qc)
                nb = len(blocks)
                done_kk = set()
                staged_j = 0
                need = [set(kk for kk, _, _, _ in plans[qc]
                            if kk <= qc * 4 + j) for j in range(4)]
                osb = obuf.tile([128, 4, 2 * VW], F32, tag="osb", name="osb")
                p_tiles = {}
                for bi, (kk, c0, W, dirty) in enumerate(blocks):
                    eng = assign[(qc, kk)]
                    s_ps = spool.tile([128, 2 * QBLK], F32, tag="s")
                    p_t = ppool.tile([128, 2 * QBLK], BF16, tag="p")
                    p_tiles[kk] = (p_t, c0, W)
                    q0 = qc * QBLK + c0
                    for h in range(2):
                        nc.tensor.matmul(
                            s_ps[:, h * QBLK:h * QBLK + W],
                            lhsT=ktile[64 * h:64 * h + 64,
                                       kk * KBLK:(kk + 1) * KBLK],
                            rhs=qtile[64 * h:64 * h + 64, q0:q0 + W],
                            start=True, stop=True)
                    if W == QBLK:
                        s_in = s_ps[:, 0:2 * QBLK]
                        p_out = p_t[:, 0:2 * QBLK]
                    else:
                        s_in = s_ps[:].rearrange(
                            "p (h w) -> p h w", h=2)[:, :, 0:W]
                        p_out = p_t[:].rearrange(
                            "p (h w) -> p h w", h=2)[:, :, 0:W]
                    if eng == 'dve':
                        if dirty:
                            m2 = comp[:, 4:4 + W].unsqueeze(1).to_broadcast(
                                [128, 2, W])
                            s3 = s_ps[:].rearrange(
                                "p (h w) -> p h w", h=2)[:, :, 0:W]
                            p3 = p_t[:].bitcast(I16).rearrange(
                                "p (h w) -> p h w", h=2)[:, :, 0:W]
                            nc.vector.scalar_tensor_tensor(
                                p3, s3, B_MAGIC, m2, ALU.add, ALU.mult)
                        else:
                            nc.vector.tensor_scalar(
                                p_t[:].bitcast(I16), s_ps[:], B_MAGIC, None,
                                ALU.add)
                    else:
                        nc.scalar.activation(p_out, s_in, EXP,
                                             scale=float(1.0 / A_SCALE))
                        if dirty:
                            md = min(W, 132)
                            m2 = comp[:, 4:4 + md].unsqueeze(1).to_broadcast(
                                [128, 2, md])
                            p3 = p_t[:].rearrange(
                                "p (h w) -> p h w", h=2)[:, :, 0:md]
                            nc.vector.tensor_mul(p3, p3, m2)
                    done_kk.add(kk)
                    while staged_j < 4 and need[staged_j] <= done_kk:
                        stage_qb(p_tiles, qc, staged_j, g, vtile, osb)
                        staged_j += 1
                    budget = max(PAV_MIN_SLICE,
                                 -(-len_pending(pending) // (nb - bi + 3)))
                    emit_pav_slice(pending, budget)
                while staged_j < 4:
                    stage_qb(p_tiles, qc, staged_j, g, vtile, osb)
                    staged_j += 1
        # final drain
        emit_pav_slice(pending, 1 << 30)
    nc.finalize()
    return nc


def len_pending(pending):
    if pending is None:
        return 0
    return sum(len(m) for _, m, _, _, _, _, _ in pending["items"])


def _make_in_maps(q4, k4, v4, n_groups):
    B, S = q4.shape[0], q4.shape[1]
    NK = S // KBLK
    comp = (np.arange(516)[None, :] >= (np.arange(128)[:, None] + 4)
            ).astype(BF)
    in_maps = []
    for c in range(N_CORES):
        qt = np.empty((n_groups, 128, S), BF)
        kt = np.empty((n_groups, 128, S), BF)
        vvv = np.empty((n_groups, 128, 2, NK * VW), BF)
        for lp in range(2 * n_groups):
            gp = c * 2 * n_groups + lp
            b, h = divmod(gp, H)
            g, half = divmod(lp, 2)
            qt[g, 64 * half:64 * half + 64] = q4[b, :, h, :].T.astype(BF)
            kt[g, 64 * half:64 * half + 64] = k4[b, :, h, :].T.astype(BF)
            vt = np.ones((128, NK, VW), np.float32)
            vt[:, :, :DH] = v4[b, :, h, :].reshape(NK, KBLK, DH).transpose(1, 0, 2)
            vvv[g, :, half, :] = vt.reshape(128, NK * VW).astype(BF)
        in_maps.append({"qt": qt, "kt": kt, "vv": vvv, "cm": comp})
    return in_maps


def _assemble(results, B, S, n_groups):
    D = H * DH
    out = np.empty((B, S, D), np.float32)
    for c in range(N_CORES):
        otc = results[c]["ot"]  # [n_groups, S, 2*VW] f32
        for lp in range(2 * n_groups):
            gp = c * 2 * n_groups + lp
            b, h = divmod(gp, H)
            g, half = divmod(lp, 2)
            blk = otc[g, :, half * VW:(half + 1) * VW].astype(np.float64)
            l = blk[:, DH]
            l = np.where(l == 0.0, 1.0, l)
            out[b, :, h * DH:(h + 1) * DH] = \
                (blk[:, :DH] / l[:, None]).astype(np.float32)
    return out


_assign_cache = None


def kernel(queries, keys, values, mask):
    global _assign_cache, LAST_RESULTS, LAST_NC, LAST_INMAPS
    B, S, D = queries.shape
    assert D == H * DH
    qs = np.ascontiguousarray(queries, dtype=np.float32) * (A_SCALE / 8.0)
    q4 = qs.reshape(B, S, H, DH)
    k4 = np.ascontiguousarray(keys, dtype=np.float32).reshape(B, S, H, DH)
    v4 = np.ascontiguousarray(values, dtype=np.float32).reshape(B, S, H, DH)
    maskb = np.asarray(mask).astype(bool)

    plans = _plan(maskb)
    _assign_cache = _assign(plans)
    n_groups = (B * H) // N_CORES // 2

    nc = _build(S, n_groups, plans, _assign_cache)
    in_maps = _make_in_maps(q4, k4, v4, n_groups)
    try:
        res = run_bass_kernel_spmd(nc, in_maps, core_ids=list(range(N_CORES)))
    except ModuleNotFoundError:
        os.environ["BASS_NEVER_TRACE"] = "1"
        res = run_bass_kernel_spmd(nc, in_maps, core_ids=list(range(N_CORES)))
    LAST_RESULTS = res
    LAST_NC = nc
    LAST_INMAPS = in_maps
    return _assemble(res.results, B, S, n_groups)
